# revision 9
# baseline (speedup 1.0000x reference)
"""Chamfer distance kernel for Trainium2 (Bass/Tile), SPMD over 8 NeuronCores.

Math (per batch b):
  dist[v,l] = ||x_v||^2 - 2 x_v.y_l + ||y_l||^2,  x=[1024,512], y=[512,512]
  out[b] = mean_v min_l dist + mean_l min_v dist

Strategy (fp8 DoubleRow pipeline, v5):
  - Data-parallel over batch: 64 batches -> 8 cores x 8 batches.
  - Host packs xs = fp8_e4m3(-2x) and ys = fp8_e4m3(y) in k-chunk layout
    [P, KC, N] (contraction on partitions). Norms of the QUANTIZED points
    ride 4 extra fp8 contraction rows in hi/lo residual encoding
      a = 64*fp8(a/64) + fp8(a - 64*fp8(a/64))   (error < +-0.5), same b
    inside a K=128 zero-padded plain-fp8 aug matmul per v-chunk (small-K
    matmuls cost ~1.6x a K=128 one on HW, so padding is a win).
  - PE per v-chunk: 2 fp8 DoubleRow matmuls (K=512, ~2 rows/cycle at
    steady state) + 1 padded aug -> PSUM holds the full dist chunk.
  - ACT: pure paired copies PSUM->SBUF fp16 (two chunks per op).
  - DVE (few, big ops): D1 = free-dim min reduce over [P,2,NL] (pair 0)
    and [P,6,NL] (rest); D2 = running elementwise fp16 min into a
    [P,2,NL] accumulator pair; merge converts to fp32.
  - D2 finale per batch (software-pipelined one batch late so the PE
    never stalls on it): 4 fp32 PE transposes + one [P,4,P] min reduce.
  - End: cross-batch reduce of d1/d2 accumulators, ones-matmul
    cross-partition sum, scale, DMA [1, 8] per core.
"""

import numpy as np

N_CORES = 8
B = 8          # batches per core
D = 512        # feature dim
NV = 1024      # video clips
NL = 512       # language tokens
P = 128        # partitions
KC = D // P    # contraction chunks = 4
MC = NV // P   # v chunks = 8

_CACHE = {}


def _build_bass():
    import concourse.bass as bass
    import concourse.mybir as mybir
    import concourse.tile as tile
    from concourse import bacc
    from concourse.masks import make_identity

    f32 = mybir.dt.float32
    f16 = mybir.dt.float16
    f8 = mybir.dt.float8e4
    ALU = mybir.AluOpType
    AX = mybir.AxisListType
    DR = mybir.MatmulPerfMode.DoubleRow

    nc = bacc.Bacc(None)
    xs_h = nc.declare_dram_parameter("xs", [B, P, KC, NV], f8, isOutput=False)
    ys_h = nc.declare_dram_parameter("ys", [B, P, KC, NL], f8, isOutput=False)
    as_h = nc.declare_dram_parameter("as_", [B, P, NV], f8, isOutput=False)
    am_h = nc.declare_dram_parameter("am", [B, P, NL], f8, isOutput=False)
    out_h = nc.declare_dram_parameter("out", [1, B], f32, isOutput=True)

    with tile.TileContext(nc) as tc:
        with (
            tc.tile_pool(name="const", bufs=1) as cpool,
            tc.tile_pool(name="io", bufs=3) as io,
            tc.tile_pool(name="work", bufs=2) as work,
            tc.tile_pool(name="acc", bufs=1) as accp,
            tc.tile_pool(name="ps", bufs=3, space="PSUM") as ps,
            tc.tile_pool(name="psn", bufs=1, space="PSUM") as psn,
        ):
            ident16 = cpool.tile([P, P], f16, tag="ident")
            make_identity(nc, ident16)
            ones_f32 = cpool.tile([P, 1], f32, tag="onesf")
            nc.vector.memset(ones_f32, 1.0)

            d1call = accp.tile([P, B, MC], f16, tag="d1call")
            d2call = accp.tile([P, B, KC], f16, tag="d2call")
            d1sums = accp.tile([P, B], f32, tag="d1s")
            d2sums = accp.tile([P, B], f32, tag="d2s")
            dall = accp.tile([P, B], f32, tag="dall")
            out_sb = accp.tile([1, B], f32, tag="osb")

            finale_args = [None] * B

            def issue_finale(bi):
                rt2p, rtfp = finale_args[bi]
                # Merge accumulator pair (all-fp16 keeps DVE in 2x mode).
                nc.vector.tensor_tensor(
                    out=rtfp, in0=rt2p[:, 0, :], in1=rt2p[:, 1, :], op=ALU.min
                )
                t_ps = psn.tile([P, KC, P], f16, tag="t", bufs=1)
                for j in range(KC):
                    nc.tensor.transpose(
                        out=t_ps[:, j, :],
                        in_=rtfp[:, j * P : (j + 1) * P],
                        identity=ident16,
                    )
                nc.vector.tensor_reduce(
                    out=d2call[:, bi, :], in_=t_ps, axis=AX.X, op=ALU.min
                )

            for b in range(B):
                xs_t = io.tile([P, KC, NV], f8, tag="xs")
                ys_t = io.tile([P, KC, NL], f8, tag="ys")
                as_t = io.tile([P, NV], f8, tag="as")
                am_t = io.tile([P, NL], f8, tag="am")
                nc.sync.dma_start(out=xs_t[:, :2], in_=xs_h[b, :, :2])
                nc.sync.dma_start(out=xs_t[:, 2:], in_=xs_h[b, :, 2:])
                nc.sync.dma_start(out=ys_t, in_=ys_h[b])
                nc.sync.dma_start(out=as_t, in_=as_h[b])
                nc.sync.dma_start(out=am_t, in_=am_h[b])

                c_all = work.tile([P, MC, NL], f16, tag="call", bufs=2)
                rt2 = work.tile([P, 2, NL], f16, tag="rt2", bufs=2)
                rtf = work.tile([P, NL], f16, tag="rtf", bufs=2)
                finale_args[b] = (rt2, rtf)

                for pr in range(MC // 2):
                    pm2 = ps.tile([P, 2, NL], f32, tag="pm", bufs=3)
                    for j in range(2):
                        m = 2 * pr + j
                        pm = pm2[:, j, :]
                        for kt2 in range(2):
                            nc.tensor.matmul(
                                out=pm,
                                lhsT=xs_t[:, 2 * kt2 : 2 * kt2 + 2, m * P : (m + 1) * P],
                                rhs=ys_t[:, 2 * kt2 : 2 * kt2 + 2, :],
                                start=(kt2 == 0),
                                stop=False,
                                perf_mode=DR,
                            )
                        # norm aug (a_v + b_l), K=128 zero-padded plain fp8.
                        nc.tensor.matmul(
                            out=pm,
                            lhsT=as_t[:, m * P : (m + 1) * P],
                            rhs=am_t,
                            start=False,
                            stop=True,
                        )
                    if pr == 0:
                        # Pair 0 lands directly in the D2 accumulator; D1 for
                        # chunks 0-1 reads it before any D2 min overwrites.
                        nc.scalar.copy(out=rt2, in_=pm2)
                        nc.vector.tensor_reduce(
                            out=d1call[:, b, 0:2], in_=rt2, axis=AX.X, op=ALU.min
                        )
                    else:
                        cpair = c_all[:, 2 * pr : 2 * pr + 2, :]
                        nc.scalar.copy(out=cpair, in_=pm2)
                        nc.vector.tensor_reduce(
                            out=d1call[:, b, 2 * pr : 2 * pr + 2],
                            in_=cpair,
                            axis=AX.X,
                            op=ALU.min,
                        )
                        nc.vector.tensor_tensor(
                            out=rt2, in0=cpair, in1=rt2, op=ALU.min
                        )
                    if pr == 1 and b > 0:
                        # Software-pipelined finale of the previous batch:
                        # its DVE merge is ready by now, so the PE transposes
                        # slot between this batch's matmul pairs stall-free.
                        issue_finale(b - 1)

            issue_finale(B - 1)

            # Cross-batch: d1sums/d2sums [P, B] <- min-chunk sums.
            nc.vector.tensor_reduce(
                out=d1sums, in_=d1call, axis=AX.X, op=ALU.add
            )
            nc.vector.tensor_reduce(
                out=d2sums, in_=d2call, axis=AX.X, op=ALU.add
            )
            # out[b] = (sum_p d1sums + 2 * sum_p d2sums) / 1024
            nc.vector.scalar_tensor_tensor(
                out=dall,
                in0=d2sums,
                scalar=2.0,
                in1=d1sums,
                op0=ALU.mult,
                op1=ALU.add,
            )
            f_ps = psn.tile([1, B], f32, tag="fin", bufs=1)
            nc.tensor.matmul(
                out=f_ps, lhsT=ones_f32, rhs=dall, start=True, stop=True
            )
            nc.scalar.mul(out=out_sb, in_=f_ps, mul=1.0 / NV)
            nc.sync.dma_start(out=out_h[:], in_=out_sb)

    nc.finalize()
    return nc


def _get_bass():
    if "nc" not in _CACHE:
        _CACHE["nc"] = _build_bass()
    return _CACHE["nc"]


def _run(in_maps, trace=False):
    from concourse.bass_utils import run_bass_kernel_spmd

    nc = _get_bass()
    return run_bass_kernel_spmd(nc, in_maps, list(range(N_CORES)), trace=trace)


def make_in_maps(video_feat, lang_feat):
    import ml_dtypes

    f8 = ml_dtypes.float8_e4m3
    video = np.asarray(video_feat, dtype=np.float32)
    lang = np.asarray(lang_feat, dtype=np.float32)
    assert video.shape == (N_CORES * B, NV, D), video.shape
    assert lang.shape == (N_CORES * B, NL, D), lang.shape
    NB = N_CORES * B

    # Quantize once for all batches.
    xs8 = (-2.0 * video).astype(f8)                      # [64, NV, D]
    ys8 = lang.astype(f8)                                # [64, NL, D]
    xsf = xs8.astype(np.float32)
    ysf = ys8.astype(np.float32)
    a = np.einsum("bvd,bvd->bv", xsf, xsf) / 4.0         # ||x_q||^2  [64, NV]
    bn = np.einsum("bld,bld->bl", ysf, ysf)              # ||y_q||^2  [64, NL]

    def hi_lo(v):
        hi = (v / 64.0).astype(f8)
        lo = (v - 64.0 * hi.astype(np.float32)).astype(f8)
        return hi, lo

    a_hi, a_lo = hi_lo(a)
    b_hi, b_lo = hi_lo(bn)

    # aug stationary [64, P, NV]: rows (64s, a_hi, 1s, a_lo), rest zero.
    as_dev = np.zeros((NB, P, NV), f8)
    as_dev[:, 0, :] = np.float32(64.0)
    as_dev[:, 1, :] = a_hi
    as_dev[:, 2, :] = np.float32(1.0)
    as_dev[:, 3, :] = a_lo
    # aug moving [64, P, NL]: rows (b_hi, 64s, b_lo, 1s), rest zero.
    am_dev = np.zeros((NB, P, NL), f8)
    am_dev[:, 0, :] = b_hi
    am_dev[:, 1, :] = np.float32(64.0)
    am_dev[:, 2, :] = b_lo
    am_dev[:, 3, :] = np.float32(1.0)

    # Device layouts: [P, KC, N] with element (p, kt, n) = op[n, kt*P+p].
    xs_dev = np.ascontiguousarray(
        xs8.reshape(NB, NV, KC, P).transpose(0, 3, 2, 1)
    )  # [64, P, KC, NV]
    ys_dev = np.ascontiguousarray(
        ys8.reshape(NB, NL, KC, P).transpose(0, 3, 2, 1)
    )  # [64, P, KC, NL]

    in_maps = []
    for c in range(N_CORES):
        sl = slice(c * B, (c + 1) * B)
        in_maps.append(
            {
                "xs": xs_dev[sl],
                "ys": ys_dev[sl],
                "as_": as_dev[sl],
                "am": am_dev[sl],
            }
        )
    return in_maps


def kernel(video_feat, lang_feat):
    in_maps = make_in_maps(video_feat, lang_feat)
    res = _run(in_maps, trace=False)
    outs = [res.results[c]["out"].reshape(-1) for c in range(N_CORES)]
    return np.concatenate(outs).astype(np.float32)


# revision 10
# speedup vs baseline: 1.1666x; 1.1666x over previous
"""Chamfer distance kernel for Trainium2 (Bass/Tile), SPMD over 8 NeuronCores.

Math (per batch b):
  dist[v,l] = ||x_v||^2 - 2 x_v.y_l + ||y_l||^2,  x=[1024,512], y=[512,512]
  out[b] = mean_v min_l dist + mean_l min_v dist

Strategy (fp8 DoubleRow pipeline, v5):
  - Data-parallel over batch: 64 batches -> 8 cores x 8 batches.
  - Host packs xs = fp8_e4m3(-2x) and ys = fp8_e4m3(y) in k-chunk layout
    [P, KC, N] (contraction on partitions). Norms of the QUANTIZED points
    ride 4 extra fp8 contraction rows in hi/lo residual encoding
      a = 64*fp8(a/64) + fp8(a - 64*fp8(a/64))   (error < +-0.5), same b
    inside a K=128 zero-padded plain-fp8 aug matmul per v-chunk (small-K
    matmuls cost ~1.6x a K=128 one on HW, so padding is a win).
  - PE per v-chunk: 2 fp8 DoubleRow matmuls (K=512, ~2 rows/cycle at
    steady state) + 1 padded aug -> PSUM holds the full dist chunk.
  - ACT: pure paired copies PSUM->SBUF fp16 (two chunks per op).
  - DVE (few, big ops): D1 = free-dim min reduce over [P,2,NL] (pair 0)
    and [P,6,NL] (rest); D2 = running elementwise fp16 min into a
    [P,2,NL] accumulator pair; merge converts to fp32.
  - D2 finale per batch (software-pipelined one batch late so the PE
    never stalls on it): 4 fp32 PE transposes + one [P,4,P] min reduce.
  - End: cross-batch reduce of d1/d2 accumulators, ones-matmul
    cross-partition sum, scale, DMA [1, 8] per core.
"""

import numpy as np

N_CORES = 8
B = 8          # batches per core
D = 512        # feature dim
NV = 1024      # video clips
NL = 512       # language tokens
P = 128        # partitions
KC = D // P    # contraction chunks = 4
MC = NV // P   # v chunks = 8

_CACHE = {}


def _build_bass():
    import concourse.bass as bass
    import concourse.mybir as mybir
    import concourse.tile as tile
    from concourse import bacc
    from concourse.masks import make_identity

    f32 = mybir.dt.float32
    f16 = mybir.dt.float16
    f8 = mybir.dt.float8e4
    ALU = mybir.AluOpType
    AX = mybir.AxisListType
    DR = mybir.MatmulPerfMode.DoubleRow

    nc = bacc.Bacc(None)
    xs_h = nc.declare_dram_parameter("xs", [B, P, KC, NV], f8, isOutput=False)
    ys_h = nc.declare_dram_parameter("ys", [B, P, KC, NL], f8, isOutput=False)
    as_h = nc.declare_dram_parameter("as_", [B, P, NV], f8, isOutput=False)
    am_h = nc.declare_dram_parameter("am", [B, P, NL], f8, isOutput=False)
    out_h = nc.declare_dram_parameter("out", [1, B], f32, isOutput=True)

    with tile.TileContext(nc) as tc:
        with (
            tc.tile_pool(name="const", bufs=1) as cpool,
            tc.tile_pool(name="io", bufs=3) as io,
            tc.tile_pool(name="work", bufs=2) as work,
            tc.tile_pool(name="acc", bufs=1) as accp,
            tc.tile_pool(name="ps", bufs=3, space="PSUM") as ps,
            tc.tile_pool(name="psn", bufs=1, space="PSUM") as psn,
        ):
            ident16 = cpool.tile([P, P], f16, tag="ident")
            make_identity(nc, ident16)
            ones_f32 = cpool.tile([P, 1], f32, tag="onesf")
            nc.vector.memset(ones_f32, 1.0)

            d1call = accp.tile([P, B, MC], f16, tag="d1call")
            d2call = accp.tile([P, B, KC], f16, tag="d2call")
            d1sums = accp.tile([P, B], f32, tag="d1s")
            d2sums = accp.tile([P, B], f32, tag="d2s")
            dall = accp.tile([P, B], f32, tag="dall")
            out_sb = accp.tile([1, B], f32, tag="osb")

            finale_args = [None] * B

            def issue_finale(bi):
                rt2p, rtfp = finale_args[bi]
                # Merge accumulator pair (all-fp16 keeps DVE in 2x mode).
                nc.vector.tensor_tensor(
                    out=rtfp, in0=rt2p[:, 0, :], in1=rt2p[:, 1, :], op=ALU.min
                )
                t_ps = psn.tile([P, KC, P], f16, tag="t", bufs=1)
                for j in range(KC):
                    nc.tensor.transpose(
                        out=t_ps[:, j, :],
                        in_=rtfp[:, j * P : (j + 1) * P],
                        identity=ident16,
                    )
                nc.vector.tensor_reduce(
                    out=d2call[:, bi, :], in_=t_ps, axis=AX.X, op=ALU.min
                )

            for b in range(B):
                xs_t = io.tile([P, KC, NV], f8, tag="xs")
                ys_t = io.tile([P, KC, NL], f8, tag="ys")
                as_t = io.tile([P, NV], f8, tag="as")
                am_t = io.tile([P, NL], f8, tag="am")
                nc.sync.dma_start(out=xs_t[:, :2], in_=xs_h[b, :, :2])
                nc.sync.dma_start(out=xs_t[:, 2:], in_=xs_h[b, :, 2:])
                nc.sync.dma_start(out=ys_t, in_=ys_h[b])
                nc.sync.dma_start(out=as_t, in_=as_h[b])
                nc.sync.dma_start(out=am_t, in_=am_h[b])

                c_all = work.tile([P, MC, NL], f16, tag="call", bufs=2)
                rt2 = work.tile([P, 2, NL], f16, tag="rt2", bufs=2)
                rtf = work.tile([P, NL], f16, tag="rtf", bufs=2)
                finale_args[b] = (rt2, rtf)

                for pr in range(MC // 2):
                    pm2 = ps.tile([P, 2, NL], f32, tag="pm", bufs=3)
                    for j in range(2):
                        m = 2 * pr + j
                        pm = pm2[:, j, :]
                        for kt2 in range(2):
                            nc.tensor.matmul(
                                out=pm,
                                lhsT=xs_t[:, 2 * kt2 : 2 * kt2 + 2, m * P : (m + 1) * P],
                                rhs=ys_t[:, 2 * kt2 : 2 * kt2 + 2, :],
                                start=(kt2 == 0),
                                stop=False,
                                perf_mode=DR,
                            )
                        # norm aug (a_v + b_l), K=128 zero-padded plain fp8.
                        nc.tensor.matmul(
                            out=pm,
                            lhsT=as_t[:, m * P : (m + 1) * P],
                            rhs=am_t,
                            start=False,
                            stop=True,
                        )
                    if pr == 0:
                        # Pair 0 lands directly in the D2 accumulator; D1 for
                        # chunks 0-1 reads it before any D2 min overwrites.
                        nc.scalar.copy(out=rt2, in_=pm2)
                        nc.vector.tensor_reduce(
                            out=d1call[:, b, 0:2], in_=rt2, axis=AX.X, op=ALU.min
                        )
                    else:
                        cpair = c_all[:, 2 * pr : 2 * pr + 2, :]
                        nc.scalar.copy(out=cpair, in_=pm2)
                        nc.vector.tensor_tensor(
                            out=rt2, in0=cpair, in1=rt2, op=ALU.min
                        )
                    if pr == 3 and b > 0:
                        # Software-pipelined finale of the previous batch:
                        # issued after ALL of this batch's matmul pairs so the
                        # PE never stalls waiting on the DVE merge.
                        issue_finale(b - 1)

                nc.vector.tensor_reduce(
                    out=d1call[:, b, 2:MC],
                    in_=c_all[:, 2:MC, :],
                    axis=AX.X,
                    op=ALU.min,
                )

            issue_finale(B - 1)

            # Cross-batch: d1sums/d2sums [P, B] <- min-chunk sums.
            nc.vector.tensor_reduce(
                out=d1sums, in_=d1call, axis=AX.X, op=ALU.add
            )
            nc.vector.tensor_reduce(
                out=d2sums, in_=d2call, axis=AX.X, op=ALU.add
            )
            # out[b] = (sum_p d1sums + 2 * sum_p d2sums) / 1024
            nc.vector.scalar_tensor_tensor(
                out=dall,
                in0=d2sums,
                scalar=2.0,
                in1=d1sums,
                op0=ALU.mult,
                op1=ALU.add,
            )
            f_ps = psn.tile([1, B], f32, tag="fin", bufs=1)
            nc.tensor.matmul(
                out=f_ps, lhsT=ones_f32, rhs=dall, start=True, stop=True
            )
            nc.scalar.mul(out=out_sb, in_=f_ps, mul=1.0 / NV)
            nc.sync.dma_start(out=out_h[:], in_=out_sb)

    nc.finalize()
    return nc


def _get_bass():
    if "nc" not in _CACHE:
        _CACHE["nc"] = _build_bass()
    return _CACHE["nc"]


def _run(in_maps, trace=False):
    from concourse.bass_utils import run_bass_kernel_spmd

    nc = _get_bass()
    return run_bass_kernel_spmd(nc, in_maps, list(range(N_CORES)), trace=trace)


def make_in_maps(video_feat, lang_feat):
    import ml_dtypes

    f8 = ml_dtypes.float8_e4m3
    video = np.asarray(video_feat, dtype=np.float32)
    lang = np.asarray(lang_feat, dtype=np.float32)
    assert video.shape == (N_CORES * B, NV, D), video.shape
    assert lang.shape == (N_CORES * B, NL, D), lang.shape
    NB = N_CORES * B

    # Quantize once for all batches.
    xs8 = (-2.0 * video).astype(f8)                      # [64, NV, D]
    ys8 = lang.astype(f8)                                # [64, NL, D]
    xsf = xs8.astype(np.float32)
    ysf = ys8.astype(np.float32)
    a = np.einsum("bvd,bvd->bv", xsf, xsf) / 4.0         # ||x_q||^2  [64, NV]
    bn = np.einsum("bld,bld->bl", ysf, ysf)              # ||y_q||^2  [64, NL]

    def hi_lo(v):
        hi = (v / 64.0).astype(f8)
        lo = (v - 64.0 * hi.astype(np.float32)).astype(f8)
        return hi, lo

    a_hi, a_lo = hi_lo(a)
    b_hi, b_lo = hi_lo(bn)

    # aug stationary [64, P, NV]: rows (64s, a_hi, 1s, a_lo), rest zero.
    as_dev = np.zeros((NB, P, NV), f8)
    as_dev[:, 0, :] = np.float32(64.0)
    as_dev[:, 1, :] = a_hi
    as_dev[:, 2, :] = np.float32(1.0)
    as_dev[:, 3, :] = a_lo
    # aug moving [64, P, NL]: rows (b_hi, 64s, b_lo, 1s), rest zero.
    am_dev = np.zeros((NB, P, NL), f8)
    am_dev[:, 0, :] = b_hi
    am_dev[:, 1, :] = np.float32(64.0)
    am_dev[:, 2, :] = b_lo
    am_dev[:, 3, :] = np.float32(1.0)

    # Device layouts: [P, KC, N] with element (p, kt, n) = op[n, kt*P+p].
    xs_dev = np.ascontiguousarray(
        xs8.reshape(NB, NV, KC, P).transpose(0, 3, 2, 1)
    )  # [64, P, KC, NV]
    ys_dev = np.ascontiguousarray(
        ys8.reshape(NB, NL, KC, P).transpose(0, 3, 2, 1)
    )  # [64, P, KC, NL]

    in_maps = []
    for c in range(N_CORES):
        sl = slice(c * B, (c + 1) * B)
        in_maps.append(
            {
                "xs": xs_dev[sl],
                "ys": ys_dev[sl],
                "as_": as_dev[sl],
                "am": am_dev[sl],
            }
        )
    return in_maps


def kernel(video_feat, lang_feat):
    in_maps = make_in_maps(video_feat, lang_feat)
    res = _run(in_maps, trace=False)
    outs = [res.results[c]["out"].reshape(-1) for c in range(N_CORES)]
    return np.concatenate(outs).astype(np.float32)
